# revision 24
# baseline (speedup 1.0000x reference)
"""Trainium2 Bass kernel for nn_FRAP_move (FRAP traffic-signal Q-network).

Strategy
--------
Pure data parallelism over the batch dim (8 cores x 8192 rows). On each core
everything is computed feature-major: features live on SBUF partitions, a
batch tile of T=512 rows is the moving free dimension of every matmul.

All network parameters are tiny, and phase2movements / comp_mask are 0/1
masks fixed across the batch, so the whole [B,P,M,*] computation collapses
on the host into a handful of structured matrices that are applied on-device
as TensorE matmuls in float32r (fp32 bits, ~12-bit mantissa PE mode; streams
at ~2 cycles/column but keeps rel err ~6e-4 end to end).

The input daT[40, bc] carries states^T in rows 0..12 and a host-computed
onehot(act) in rows 32..39 (base-32 aligned for matmul operand slicing):

  daT --MM-A--> dW[k]*dem[m] --sigmoid+bias--> s1[48,T]
  s1,oh --MM-D (PSUM accum)--> pre[(m,h) 192,T] --relu--> relu1
  relu1,oh --MM-F (PSUM accum)--> agg[(p,h) 128,T]
  agg --MM-G--> rot_pre[(pair,o) 120,T] per 6-pair group --relu+bias (DVE)-->
      --MM-I (block-diag hid_W*rel)--> --relu+bias (ACT)--> --MM-J--> q[8,T]

The pairwise relation factor rel[i,j] takes only two values (comp_mask is
0/1), folded into the MM-I weights on the host.
"""

import os
import sys
from contextlib import ExitStack

import numpy as np

for _p in ("/opt/trn_rl_repo", "/root/.axon_site/_ro/trn_rl_repo"):
    if os.path.isdir(_p) and _p not in sys.path:
        sys.path.append(_p)

import concourse.bass as bass
import concourse.mybir as mybir
import concourse.tile as tile
from concourse.bass_utils import run_bass_kernel_spmd

F32 = mybir.dt.float32
F32R = mybir.dt.float32r
BF16 = mybir.dt.bfloat16
AF = mybir.ActivationFunctionType
ALU = mybir.AluOpType

B = 65536
NCORES = 8
BC = B // NCORES  # 8192 per core
T = 512           # batch tile (matmul moving free dim)

PAIRS = [(i, j) for i in range(8) for j in range(8) if j != i]
GROUPS = [PAIRS[g * 6:(g + 1) * 6] for g in range(9)] + [PAIRS[54:]]
GROUP_ROWS = [len(g) * 20 for g in GROUPS]           # [120]*9 + [40]
GROUP_OFF = np.cumsum([0] + GROUP_ROWS).tolist()     # offsets into 1120

CONST_SHAPES = {
    "cLA": (13, 48),
    "cDB": (48, 1),
    "cLDs1": (48, 192),
    "cLDoh": (40, 192),
    "cLFLO": (96, 128),
    "cLFHI": (96, 128),
    "cLFOH": (40, 128),
    "cLG": (128, 1120),
    "cLI": (120, 1120),
    "cLJ": (120, 80),
    "cLCB": (120, 1),
    "cHB": (120, 1),
    "cQB": (8, 1),
}
# matmul operands live in float32r (PE full-rate fp32 mode, ~12 mantissa bits)
F32R_CONSTS = {"cLA", "cLDs1", "cLDoh", "cLFLO", "cLFHI", "cLFOH",
               "cLG", "cLI", "cLJ"}
BF16_CONSTS = set()


def round_f32r(a):
    """Round fp32 array to the fp32r grid (12-bit mantissa, round-to-nearest)."""
    u = np.ascontiguousarray(a, np.float32).view(np.uint32)
    r = ((u.astype(np.uint64) + 0x800) & 0xFFFFF000).astype(np.uint32)
    return r.view(np.float32)

LAST_RESULTS = None
_PROGRAM_CACHE = {}


def _sigmoid(x):
    return 1.0 / (1.0 + np.exp(-x))


def _relu(x):
    return np.maximum(x, 0.0)


def build_consts(inputs):
    """Host-side fold of all parameters into the structured device matrices."""
    f32 = np.float32
    inp = {k: np.asarray(v) for k, v in inputs.items()}
    dW = inp["d_W"].astype(f32)[:, 0]
    db = inp["d_b"].astype(f32)
    lane_W = inp["lane_W"].astype(f32)
    lane_b = inp["lane_b"].astype(f32)
    Wd, We = lane_W[:, :4], lane_W[:, 4:]
    p_emb = inp["p_emb"].astype(f32)
    e0, e1 = _sigmoid(p_emb[0]), _sigmoid(p_emb[1])
    v0, v1 = We @ e0, We @ e1
    dv = v1 - v0
    u0 = Wd @ _sigmoid(db)
    r0 = _relu(u0 + v0 + lane_b)
    r1 = _relu(u0 + v1 + lane_b)
    drr = r1 - r0
    p2m = inp["phase2movements"].astype(f32)
    np_p = p2m.sum(1)
    lane_conv_W = inp["lane_conv_W"].astype(f32)
    W1, W2 = lane_conv_W[:, :16], lane_conv_W[:, 16:]
    lcb = inp["lane_conv_b"].astype(f32)
    relv = [
        _relu(inp["rel_conv_W"].astype(f32) @ _relu(inp["rel_emb"].astype(f32)[k])
              + inp["rel_conv_b"].astype(f32))
        for k in (0, 1)
    ]
    hid_W = inp["hid_W"].astype(f32)
    H = [hid_W * relv[k][None, :] for k in (0, 1)]
    hb = inp["hid_b"].astype(f32)
    mW = inp["merge_W"].astype(f32)[0]
    mb = float(inp["merge_b"].astype(f32)[0])
    comp = inp["comp_mask"].astype(np.int64)

    C = {}
    # MM-A: da[13,T] (row0=act, rows1..12=dem) -> dW[k]*dem[m] packed (k,m)
    LA = np.zeros((13, 48), f32)
    for k in range(4):
        for m in range(12):
            LA[1 + m, k * 12 + m] = dW[k]
    C["cLA"] = LA
    dbc = np.zeros((48, 1), f32)
    for k in range(4):
        dbc[k * 12:(k + 1) * 12, 0] = db[k]
    C["cDB"] = dbc

    # MM-D: s1x[56,T] = [s1 (k,m); onehot(8)] -> pre[(m,h) 192]
    LB = np.zeros((8, 13), f32)
    LB[:, :12] = p2m
    LB[:, 12] = 1.0
    LD_s1 = np.zeros((48, 192), f32)
    for k in range(4):
        for m in range(12):
            LD_s1[k * 12 + m, m * 16:(m + 1) * 16] = Wd[:, k]
    LD_c = np.zeros((13, 192), f32)
    for m in range(12):
        LD_c[m, m * 16:(m + 1) * 16] = dv
    LD_c[12, :] = np.tile(v0 + lane_b, 12)
    C["cLDs1"] = LD_s1
    CLDoh = np.zeros((40, 192), f32)
    CLDoh[32:40] = LB @ LD_c
    C["cLDoh"] = CLDoh

    # MM-F: relu1[(m,h) 192] + onehot -> agg[(p,h) 128]
    LF_relu = np.zeros((192, 128), f32)
    for m in range(12):
        for p in range(8):
            if p2m[p, m] > 0.5:
                for h in range(16):
                    LF_relu[m * 16 + h, p * 16 + h] = 1.0
    LF_c = np.zeros((13, 128), f32)
    for p in range(8):
        for m in range(12):
            LF_c[m, p * 16:(p + 1) * 16] = (1.0 - p2m[p, m]) * drr
        LF_c[12, p * 16:(p + 1) * 16] = (12.0 - np_p[p]) * r0
    C["cLFLO"] = LF_relu[:96].copy()
    C["cLFHI"] = LF_relu[96:].copy()
    CLFOH = np.zeros((40, 128), f32)
    CLFOH[32:40] = LB @ LF_c
    C["cLFOH"] = CLFOH

    # pair stage
    LG = np.zeros((128, 1120), f32)
    LI = np.zeros((120, 1120), f32)
    LJ = np.zeros((120, 80), f32)
    for g, gp in enumerate(GROUPS):
        off = GROUP_OFF[g]
        for kk, (i, j) in enumerate(gp):
            col0 = off + kk * 20
            LG[i * 16:(i + 1) * 16, col0:col0 + 20] += W1.T
            LG[j * 16:(j + 1) * 16, col0:col0 + 20] += W2.T
            jj = [x for x in range(8) if x != i].index(j)
            mk = int(comp[i, jj])
            LI[kk * 20:(kk + 1) * 20, col0:col0 + 20] = H[mk].T
            LJ[kk * 20:(kk + 1) * 20, g * 8 + i] = mW
    C["cLG"] = LG
    C["cLI"] = LI
    C["cLJ"] = LJ
    C["cLCB"] = np.tile(lcb, 6)[:, None].astype(f32)
    C["cHB"] = np.tile(hb, 6)[:, None].astype(f32)
    C["cQB"] = np.full((8, 1), 7.0 * mb, f32)
    import ml_dtypes
    for k, v in C.items():
        assert v.shape == CONST_SHAPES[k], (k, v.shape)
        if k in F32R_CONSTS:
            C[k] = round_f32r(v)
        elif k in BF16_CONSTS:
            C[k] = np.ascontiguousarray(v.astype(ml_dtypes.bfloat16))
        else:
            C[k] = np.ascontiguousarray(v, f32)
    return C


def _emit(nc, tc, ctx, daT, qT, cs, bc):
    """Emit the per-core program: bc batch rows in tiles of T."""
    nt = bc // T
    ts = bass.ts

    consts = ctx.enter_context(tc.tile_pool(name="consts", bufs=1))
    sb = ctx.enter_context(tc.tile_pool(name="sb", bufs=3))
    sbp = ctx.enter_context(tc.tile_pool(name="sbp", bufs=3))
    ps1 = ctx.enter_context(tc.tile_pool(name="ps1", bufs=1, space="PSUM"))
    ps2 = ctx.enter_context(tc.tile_pool(name="ps2", bufs=2, space="PSUM"))

    c = {}
    for name, shape in CONST_SHAPES.items():
        dt_ = (F32R if name in F32R_CONSTS
               else BF16 if name in BF16_CONSTS else F32)
        t_ = consts.tile(list(shape), dt_, tag=name)
        nc.sync.dma_start(t_[:], cs[name].ap())
        c[name] = t_

    for t in range(nt):
        da = sb.tile([40, T], F32R, tag="da")
        nc.sync.dma_start(da[:], daT.ap()[:, ts(t, T)])

        ps48 = ps2.tile([48, T], F32, tag="ps_misc")
        nc.tensor.matmul(ps48[:], c["cLA"][:], da[0:13, :], start=True, stop=True)
        s1 = sb.tile([48, T], F32R, tag="s1")
        nc.scalar.activation(s1[:], ps48[:], AF.Sigmoid, bias=c["cDB"][:])
        oh = da[32:40, :]  # host-computed onehot rows

        pre_lo = ps2.tile([96, T], F32, tag="ps_misc")
        nc.tensor.matmul(pre_lo[:], c["cLDs1"][:, 0:96], s1[:],
                         start=True, stop=False)
        nc.tensor.matmul(pre_lo[:], c["cLDoh"][32:40, 0:96], oh,
                         start=False, stop=True)
        pre_hi = ps2.tile([96, T], F32, tag="ps_misc")
        nc.tensor.matmul(pre_hi[:], c["cLDs1"][:, 96:192], s1[:],
                         start=True, stop=False)
        nc.tensor.matmul(pre_hi[:], c["cLDoh"][32:40, 96:192], oh,
                         start=False, stop=True)
        r1lo = sb.tile([96, T], F32R, tag="r1lo")
        nc.scalar.activation(r1lo[:], pre_lo[:], AF.Relu)
        r1hi = sb.tile([96, T], F32R, tag="r1hi")
        nc.scalar.activation(r1hi[:], pre_hi[:], AF.Relu)

        ps_agg = ps1.tile([128, T], F32, tag="ps_agg")
        nc.tensor.matmul(ps_agg[:], c["cLFLO"][:], r1lo[:],
                         start=True, stop=False)
        nc.tensor.matmul(ps_agg[:], c["cLFHI"][:], r1hi[:],
                         start=False, stop=False)
        nc.tensor.matmul(ps_agg[:], c["cLFOH"][32:40, :], oh,
                         start=False, stop=True)
        agg = sb.tile([128, T], F32R, tag="agg")
        nc.vector.tensor_copy(agg[:], ps_agg[:])

        ps_q = ps1.tile([8, T], F32, tag="ps_q")
        for g in range(10):
            rows = GROUP_ROWS[g]
            off = GROUP_OFF[g]
            ps_rot = ps2.tile([120, T], F32, tag="ps_rot")
            nc.tensor.matmul(ps_rot[0:rows, :], c["cLG"][:, off:off + rows],
                             agg[:], start=True, stop=True)
            rot = sbp.tile([120, T], F32R, tag="rot")
            nc.vector.tensor_scalar(rot[0:rows, :], ps_rot[0:rows, :],
                                    c["cLCB"][0:rows, :], 0.0, ALU.add, ALU.max)
            ps_comb = ps2.tile([120, T], F32, tag="ps_comb")
            nc.tensor.matmul(ps_comb[0:rows, :],
                             c["cLI"][0:rows, off:off + rows],
                             rot[0:rows, :], start=True, stop=True)
            comb = sbp.tile([120, T], F32R, tag="comb")
            nc.scalar.activation(comb[0:rows, :], ps_comb[0:rows, :], AF.Relu,
                                 bias=c["cHB"][0:rows, :])
            nc.tensor.matmul(ps_q[:], c["cLJ"][0:rows, g * 8:(g + 1) * 8],
                             comb[0:rows, :], start=(g == 0), stop=(g == 9),
                             skip_group_check=True)

        q = sb.tile([8, T], F32, tag="q")
        nc.scalar.activation(q[:], ps_q[:], AF.Identity, bias=c["cQB"][:])
        nc.sync.dma_start(qT.ap()[:, ts(t, T)], q[:])


def _strip_covered_pe_waits(nc):
    """fp32r matmuls lower to a single fused instruction that can carry only
    ONE sync wait. Tile sometimes emits a PE self-wait (psum-bank WAW)
    alongside a compute-engine wait that already transitively guarantees it
    (Tile's vector clock is not transitive across engines). Strip a matmul's
    PE wait only when another of its waits provably implies it; fail loudly
    if any matmul still carries more than one wait."""
    from collections import defaultdict

    f = nc.m.functions[0]
    sem_instrs = defaultdict(list)  # sem name -> [(cum_value_after, pe_req)]
    cum = defaultdict(int)
    for blk in f.blocks:
        for ins in blk.instructions:
            si = ins.sync_info
            if si is None:
                continue
            pe_req = 0
            for w in si.on_wait:
                if w.ant_name and w.ant_name.startswith("PE"):
                    pe_req = max(pe_req, w.wait_value)
            for u in si.on_update:
                cum[u.ant_name] += u.update_value
                sem_instrs[u.ant_name].append((cum[u.ant_name], pe_req))
    prefix = {}
    for name, lst in sem_instrs.items():
        mx = 0
        out = []
        for cv, pr in lst:
            mx = max(mx, pr)
            out.append((cv, mx))
        prefix[name] = out

    def covered(sem, val, pe_needed):
        best = 0
        for cv, mx in prefix.get(sem, []):
            if cv <= val:
                best = mx
            else:
                break
        return best >= pe_needed

    bad = []
    for blk in f.blocks:
        for ins in blk.instructions:
            if "Matmult" not in type(ins).__name__:
                continue
            si = ins.sync_info
            if si is None or len(si.on_wait) < 2:
                continue
            pe_w = [w for w in si.on_wait if w.ant_name and w.ant_name.startswith("PE")]
            others = [w for w in si.on_wait if not (w.ant_name and w.ant_name.startswith("PE"))]
            if pe_w and others:
                need = max(w.wait_value for w in pe_w)
                if any(covered(w.ant_name, w.wait_value, need) for w in others):
                    si.on_wait = others
                    ins.sync_info = si
            si = ins.sync_info
            if len(si.on_wait) > 1:
                bad.append((ins.name, [w.ant_name for w in si.on_wait]))
    if bad:
        raise RuntimeError(f"matmuls with >1 sync wait (fp32r cap): {bad[:5]}")


def build_program(bc=BC):
    if bc in _PROGRAM_CACHE:
        return _PROGRAM_CACHE[bc]
    nc = bass.Bass("TRN2", target_bir_lowering=False, debug=False)
    cs = {name: nc.dram_tensor(name, list(shape),
                               F32R if name in F32R_CONSTS
                               else BF16 if name in BF16_CONSTS else F32,
                               kind="ExternalInput")
          for name, shape in CONST_SHAPES.items()}
    daT = nc.dram_tensor("daT", [40, bc], F32R, kind="ExternalInput")
    qT = nc.dram_tensor("qT", [8, bc], F32, kind="ExternalOutput")
    with tile.TileContext(nc) as tc, ExitStack() as ctx:
        _emit(nc, tc, ctx, daT, qT, cs, bc)
    _strip_covered_pe_waits(nc)
    _PROGRAM_CACHE[bc] = nc
    return nc


def kernel(**inputs):
    global LAST_RESULTS
    states = np.ascontiguousarray(np.asarray(inputs["states"], np.float32))
    assert states.shape == (B, 13), states.shape
    C = build_consts(inputs)
    dah = np.zeros((40, B), np.float32)
    dah[0:13] = states.T
    acts = states[:, 0].astype(np.int64)
    dah[32 + np.clip(acts, 0, 7), np.arange(B)] = 1.0  # onehot(act)

    nc = build_program(BC)
    in_maps = []
    for core in range(NCORES):
        m = dict(C)
        m["daT"] = round_f32r(dah[:, core * BC:(core + 1) * BC])
        in_maps.append(m)
    res = run_bass_kernel_spmd(
        nc, in_maps, core_ids=list(range(NCORES)),
        trace=bool(os.environ.get("FRAP_TRACE")),
    )
    LAST_RESULTS = res
    q = np.concatenate([r_["qT"] for r_ in res.results], axis=1).T
    return np.ascontiguousarray(q, np.float32)


if __name__ == "__main__":
    rng = np.random.default_rng(0)
    fake = dict(
        states=np.concatenate(
            [rng.integers(0, 8, (B, 1)).astype(np.float32),
             rng.random((B, 12), np.float32)], axis=1),
        phase2movements=rng.integers(0, 2, (8, 12)),
        oshape=np.int64(8),
        comp_mask=rng.integers(0, 2, (8, 7)),
        p_emb=rng.standard_normal((2, 4), np.float32) * 0.1,
        d_W=rng.standard_normal((4, 1), np.float32) * 0.1,
        d_b=rng.standard_normal((4,), np.float32) * 0.1,
        lane_W=rng.standard_normal((16, 8), np.float32) * 0.1,
        lane_b=rng.standard_normal((16,), np.float32) * 0.1,
        lane_conv_W=rng.standard_normal((20, 32), np.float32) * 0.1,
        lane_conv_b=rng.standard_normal((20,), np.float32) * 0.1,
        rel_emb=rng.standard_normal((2, 4), np.float32) * 0.1,
        rel_conv_W=rng.standard_normal((20, 4), np.float32) * 0.1,
        rel_conv_b=rng.standard_normal((20,), np.float32) * 0.1,
        hid_W=rng.standard_normal((20, 20), np.float32) * 0.1,
        hid_b=rng.standard_normal((20,), np.float32) * 0.1,
        merge_W=rng.standard_normal((1, 20), np.float32) * 0.1,
        merge_b=rng.standard_normal((1,), np.float32) * 0.1,
    )
    out = kernel(**fake)
    print("kernel output", out.shape, out.dtype)


# revision 25
# speedup vs baseline: 1.0983x; 1.0983x over previous
"""Trainium2 Bass kernel for nn_FRAP_move (FRAP traffic-signal Q-network).

Strategy
--------
Pure data parallelism over the batch dim (8 cores x 8192 rows). On each core
everything is computed feature-major: features live on SBUF partitions, a
batch tile of T=512 rows is the moving free dimension of every matmul.

All network parameters are tiny, and phase2movements / comp_mask are 0/1
masks fixed across the batch, so the whole [B,P,M,*] computation collapses
on the host into a handful of structured matrices that are applied on-device
as TensorE matmuls in float32r (fp32 bits, ~12-bit mantissa PE mode; streams
at ~2 cycles/column but keeps rel err ~6e-4 end to end).

The input daT[40, bc] carries states^T in rows 0..12 and a host-computed
onehot(act) in rows 32..39 (base-32 aligned for matmul operand slicing):

  daT --MM-A--> dW[k]*dem[m] --sigmoid+bias--> s1[48,T]
  s1,oh --MM-D (PSUM accum)--> pre[(m,h) 192,T] --relu--> relu1
  relu1,oh --MM-F (PSUM accum)--> agg[(p,h) 128,T]
  agg --MM-G--> rot_pre[(pair,o) 120,T] per 6-pair group --relu+bias (DVE)-->
      --MM-I (block-diag hid_W*rel)--> --relu+bias (ACT)--> --MM-J--> q[8,T]

The pairwise relation factor rel[i,j] takes only two values (comp_mask is
0/1), folded into the MM-I weights on the host.
"""

import os
import sys
from contextlib import ExitStack

import numpy as np

for _p in ("/opt/trn_rl_repo", "/root/.axon_site/_ro/trn_rl_repo"):
    if os.path.isdir(_p) and _p not in sys.path:
        sys.path.append(_p)

import concourse.bass as bass
import concourse.mybir as mybir
import concourse.tile as tile
from concourse.bass_utils import run_bass_kernel_spmd

F32 = mybir.dt.float32
F32R = mybir.dt.float32r
BF16 = mybir.dt.bfloat16
AF = mybir.ActivationFunctionType
ALU = mybir.AluOpType

B = 65536
NCORES = 8
BC = B // NCORES  # 8192 per core
T = 512           # batch tile (matmul moving free dim)

PAIRS = [(i, j) for i in range(8) for j in range(8) if j != i]
GROUPS = [PAIRS[g * 6:(g + 1) * 6] for g in range(9)] + [PAIRS[54:]]
GROUP_ROWS = [len(g) * 20 for g in GROUPS]           # [120]*9 + [40]
GROUP_OFF = np.cumsum([0] + GROUP_ROWS).tolist()     # offsets into 1120

CONST_SHAPES = {
    "cLA": (13, 48),
    "cDB": (48, 1),
    "cLD": (112, 192),
    "cLFLO": (96, 128),
    "cLFHI": (96, 128),
    "cLFOH": (40, 128),
    "cLG": (128, 1120),
    "cLI": (120, 1120),
    "cLJ": (120, 80),
    "cLCB": (120, 1),
    "cHB": (120, 1),
    "cQB": (8, 1),
}
# matmul operands live in float32r (PE full-rate fp32 mode, ~12 mantissa bits)
F32R_CONSTS = {"cLA", "cLD", "cLFLO", "cLFHI", "cLFOH",
               "cLG", "cLI", "cLJ"}
BF16_CONSTS = set()


def round_f32r(a):
    """Round fp32 array to the fp32r grid (12-bit mantissa, round-to-nearest)."""
    u = np.ascontiguousarray(a, np.float32).view(np.uint32)
    r = ((u.astype(np.uint64) + 0x800) & 0xFFFFF000).astype(np.uint32)
    return r.view(np.float32)

LAST_RESULTS = None
_PROGRAM_CACHE = {}


def _sigmoid(x):
    return 1.0 / (1.0 + np.exp(-x))


def _relu(x):
    return np.maximum(x, 0.0)


def build_consts(inputs):
    """Host-side fold of all parameters into the structured device matrices."""
    f32 = np.float32
    inp = {k: np.asarray(v) for k, v in inputs.items()}
    dW = inp["d_W"].astype(f32)[:, 0]
    db = inp["d_b"].astype(f32)
    lane_W = inp["lane_W"].astype(f32)
    lane_b = inp["lane_b"].astype(f32)
    Wd, We = lane_W[:, :4], lane_W[:, 4:]
    p_emb = inp["p_emb"].astype(f32)
    e0, e1 = _sigmoid(p_emb[0]), _sigmoid(p_emb[1])
    v0, v1 = We @ e0, We @ e1
    dv = v1 - v0
    u0 = Wd @ _sigmoid(db)
    r0 = _relu(u0 + v0 + lane_b)
    r1 = _relu(u0 + v1 + lane_b)
    drr = r1 - r0
    p2m = inp["phase2movements"].astype(f32)
    np_p = p2m.sum(1)
    lane_conv_W = inp["lane_conv_W"].astype(f32)
    W1, W2 = lane_conv_W[:, :16], lane_conv_W[:, 16:]
    lcb = inp["lane_conv_b"].astype(f32)
    relv = [
        _relu(inp["rel_conv_W"].astype(f32) @ _relu(inp["rel_emb"].astype(f32)[k])
              + inp["rel_conv_b"].astype(f32))
        for k in (0, 1)
    ]
    hid_W = inp["hid_W"].astype(f32)
    H = [hid_W * relv[k][None, :] for k in (0, 1)]
    hb = inp["hid_b"].astype(f32)
    mW = inp["merge_W"].astype(f32)[0]
    mb = float(inp["merge_b"].astype(f32)[0])
    comp = inp["comp_mask"].astype(np.int64)

    C = {}
    # MM-A: da[13,T] (row0=act, rows1..12=dem) -> dW[k]*dem[m] packed (k,m)
    LA = np.zeros((13, 48), f32)
    for k in range(4):
        for m in range(12):
            LA[1 + m, k * 12 + m] = dW[k]
    C["cLA"] = LA
    dbc = np.zeros((48, 1), f32)
    for k in range(4):
        dbc[k * 12:(k + 1) * 12, 0] = db[k]
    C["cDB"] = dbc

    # MM-D: s1x[56,T] = [s1 (k,m); onehot(8)] -> pre[(m,h) 192]
    LB = np.zeros((8, 13), f32)
    LB[:, :12] = p2m
    LB[:, 12] = 1.0
    LD_s1 = np.zeros((48, 192), f32)
    for k in range(4):
        for m in range(12):
            LD_s1[k * 12 + m, m * 16:(m + 1) * 16] = Wd[:, k]
    LD_c = np.zeros((13, 192), f32)
    for m in range(12):
        LD_c[m, m * 16:(m + 1) * 16] = dv
    LD_c[12, :] = np.tile(v0 + lane_b, 12)
    # one fused MM-D operand: rows 32..39 take the onehot weights, rows
    # 64..111 the sigmoid-block weights; rhs is the da tile itself, into
    # which the sigmoid writes at partition 64
    CLD = np.zeros((112, 192), f32)
    CLD[32:40] = LB @ LD_c
    CLD[64:112] = LD_s1
    C["cLD"] = CLD

    # MM-F: relu1[(m,h) 192] + onehot -> agg[(p,h) 128]
    LF_relu = np.zeros((192, 128), f32)
    for m in range(12):
        for p in range(8):
            if p2m[p, m] > 0.5:
                for h in range(16):
                    LF_relu[m * 16 + h, p * 16 + h] = 1.0
    LF_c = np.zeros((13, 128), f32)
    for p in range(8):
        for m in range(12):
            LF_c[m, p * 16:(p + 1) * 16] = (1.0 - p2m[p, m]) * drr
        LF_c[12, p * 16:(p + 1) * 16] = (12.0 - np_p[p]) * r0
    C["cLFLO"] = LF_relu[:96].copy()
    C["cLFHI"] = LF_relu[96:].copy()
    CLFOH = np.zeros((40, 128), f32)
    CLFOH[32:40] = LB @ LF_c
    C["cLFOH"] = CLFOH

    # pair stage
    LG = np.zeros((128, 1120), f32)
    LI = np.zeros((120, 1120), f32)
    LJ = np.zeros((120, 80), f32)
    for g, gp in enumerate(GROUPS):
        off = GROUP_OFF[g]
        for kk, (i, j) in enumerate(gp):
            col0 = off + kk * 20
            LG[i * 16:(i + 1) * 16, col0:col0 + 20] += W1.T
            LG[j * 16:(j + 1) * 16, col0:col0 + 20] += W2.T
            jj = [x for x in range(8) if x != i].index(j)
            mk = int(comp[i, jj])
            LI[kk * 20:(kk + 1) * 20, col0:col0 + 20] = H[mk].T
            LJ[kk * 20:(kk + 1) * 20, g * 8 + i] = mW
    C["cLG"] = LG
    C["cLI"] = LI
    C["cLJ"] = LJ
    C["cLCB"] = np.tile(lcb, 6)[:, None].astype(f32)
    C["cHB"] = np.tile(hb, 6)[:, None].astype(f32)
    C["cQB"] = np.full((8, 1), 7.0 * mb, f32)
    import ml_dtypes
    for k, v in C.items():
        assert v.shape == CONST_SHAPES[k], (k, v.shape)
        if k in F32R_CONSTS:
            C[k] = round_f32r(v)
        elif k in BF16_CONSTS:
            C[k] = np.ascontiguousarray(v.astype(ml_dtypes.bfloat16))
        else:
            C[k] = np.ascontiguousarray(v, f32)
    return C


def _emit(nc, tc, ctx, daT, qT, cs, bc):
    """Emit the per-core program: bc batch rows in tiles of T."""
    nt = bc // T
    ts = bass.ts

    consts = ctx.enter_context(tc.tile_pool(name="consts", bufs=1))
    sb = ctx.enter_context(tc.tile_pool(name="sb", bufs=3))
    sbp = ctx.enter_context(tc.tile_pool(name="sbp", bufs=3))
    ps1 = ctx.enter_context(tc.tile_pool(name="ps1", bufs=1, space="PSUM"))
    ps2 = ctx.enter_context(tc.tile_pool(name="ps2", bufs=2, space="PSUM"))

    c = {}
    for name, shape in CONST_SHAPES.items():
        dt_ = (F32R if name in F32R_CONSTS
               else BF16 if name in BF16_CONSTS else F32)
        t_ = consts.tile(list(shape), dt_, tag=name)
        nc.sync.dma_start(t_[:], cs[name].ap())
        c[name] = t_

    for t in range(nt):
        da = sb.tile([112, T], F32R, tag="da")
        nc.sync.dma_start(da[0:64, :], daT.ap()[:, ts(t, T)])

        ps48 = ps2.tile([48, T], F32, tag="ps_misc")
        nc.tensor.matmul(ps48[:], c["cLA"][:], da[0:13, :], start=True, stop=True)
        nc.scalar.activation(da[64:112, :], ps48[:], AF.Sigmoid,
                             bias=c["cDB"][:])
        oh = da[32:40, :]  # host-computed onehot rows

        pre_lo = ps2.tile([96, T], F32, tag="ps_misc")
        nc.tensor.matmul(pre_lo[:], c["cLD"][:, 0:96], da[0:112, :],
                         start=True, stop=True)
        pre_hi = ps2.tile([96, T], F32, tag="ps_misc")
        nc.tensor.matmul(pre_hi[:], c["cLD"][:, 96:192], da[0:112, :],
                         start=True, stop=True)
        r1lo = sb.tile([96, T], F32R, tag="r1lo")
        nc.scalar.activation(r1lo[:], pre_lo[:], AF.Relu)
        r1hi = sb.tile([96, T], F32R, tag="r1hi")
        nc.scalar.activation(r1hi[:], pre_hi[:], AF.Relu)

        ps_agg = ps1.tile([128, T], F32, tag="ps_agg")
        nc.tensor.matmul(ps_agg[:], c["cLFLO"][:], r1lo[:],
                         start=True, stop=False)
        nc.tensor.matmul(ps_agg[:], c["cLFHI"][:], r1hi[:],
                         start=False, stop=False)
        nc.tensor.matmul(ps_agg[:], c["cLFOH"][32:40, :], oh,
                         start=False, stop=True)
        agg = sb.tile([128, T], F32R, tag="agg")
        nc.vector.tensor_copy(agg[:], ps_agg[:])

        ps_q = ps1.tile([8, T], F32, tag="ps_q")
        for g in range(10):
            rows = GROUP_ROWS[g]
            off = GROUP_OFF[g]
            ps_rot = ps2.tile([120, T], F32, tag="ps_rot")
            nc.tensor.matmul(ps_rot[0:rows, :], c["cLG"][:, off:off + rows],
                             agg[:], start=True, stop=True)
            rot = sbp.tile([120, T], F32R, tag="rot")
            nc.vector.tensor_scalar(rot[0:rows, :], ps_rot[0:rows, :],
                                    c["cLCB"][0:rows, :], 0.0, ALU.add, ALU.max)
            ps_comb = ps2.tile([120, T], F32, tag="ps_comb")
            nc.tensor.matmul(ps_comb[0:rows, :],
                             c["cLI"][0:rows, off:off + rows],
                             rot[0:rows, :], start=True, stop=True)
            comb = sbp.tile([120, T], F32R, tag="comb")
            nc.scalar.activation(comb[0:rows, :], ps_comb[0:rows, :], AF.Relu,
                                 bias=c["cHB"][0:rows, :])
            nc.tensor.matmul(ps_q[:], c["cLJ"][0:rows, g * 8:(g + 1) * 8],
                             comb[0:rows, :], start=(g == 0), stop=(g == 9),
                             skip_group_check=True)

        q = sb.tile([8, T], F32, tag="q")
        nc.scalar.activation(q[:], ps_q[:], AF.Identity, bias=c["cQB"][:])
        nc.sync.dma_start(qT.ap()[:, ts(t, T)], q[:])


def _strip_covered_pe_waits(nc):
    """fp32r matmuls lower to a single fused instruction that can carry only
    ONE sync wait. Tile sometimes emits a PE self-wait (psum-bank WAW)
    alongside a compute-engine wait that already transitively guarantees it
    (Tile's vector clock is not transitive across engines). Strip a matmul's
    PE wait only when another of its waits provably implies it; fail loudly
    if any matmul still carries more than one wait."""
    from collections import defaultdict

    f = nc.m.functions[0]
    sem_instrs = defaultdict(list)  # sem name -> [(cum_value_after, pe_req)]
    cum = defaultdict(int)
    for blk in f.blocks:
        for ins in blk.instructions:
            si = ins.sync_info
            if si is None:
                continue
            pe_req = 0
            for w in si.on_wait:
                if w.ant_name and w.ant_name.startswith("PE"):
                    pe_req = max(pe_req, w.wait_value)
            for u in si.on_update:
                cum[u.ant_name] += u.update_value
                sem_instrs[u.ant_name].append((cum[u.ant_name], pe_req))
    prefix = {}
    for name, lst in sem_instrs.items():
        mx = 0
        out = []
        for cv, pr in lst:
            mx = max(mx, pr)
            out.append((cv, mx))
        prefix[name] = out

    def covered(sem, val, pe_needed):
        best = 0
        for cv, mx in prefix.get(sem, []):
            if cv <= val:
                best = mx
            else:
                break
        return best >= pe_needed

    bad = []
    for blk in f.blocks:
        for ins in blk.instructions:
            if "Matmult" not in type(ins).__name__:
                continue
            si = ins.sync_info
            if si is None or len(si.on_wait) < 2:
                continue
            pe_w = [w for w in si.on_wait if w.ant_name and w.ant_name.startswith("PE")]
            others = [w for w in si.on_wait if not (w.ant_name and w.ant_name.startswith("PE"))]
            if pe_w and others:
                need = max(w.wait_value for w in pe_w)
                if any(covered(w.ant_name, w.wait_value, need) for w in others):
                    si.on_wait = others
                    ins.sync_info = si
            si = ins.sync_info
            if len(si.on_wait) > 1:
                bad.append((ins.name, [w.ant_name for w in si.on_wait]))
    if bad:
        raise RuntimeError(f"matmuls with >1 sync wait (fp32r cap): {bad[:5]}")


def build_program(bc=BC):
    if bc in _PROGRAM_CACHE:
        return _PROGRAM_CACHE[bc]
    nc = bass.Bass("TRN2", target_bir_lowering=False, debug=False)
    cs = {name: nc.dram_tensor(name, list(shape),
                               F32R if name in F32R_CONSTS
                               else BF16 if name in BF16_CONSTS else F32,
                               kind="ExternalInput")
          for name, shape in CONST_SHAPES.items()}
    daT = nc.dram_tensor("daT", [64, bc], F32R, kind="ExternalInput")
    qT = nc.dram_tensor("qT", [8, bc], F32, kind="ExternalOutput")
    with tile.TileContext(nc) as tc, ExitStack() as ctx:
        _emit(nc, tc, ctx, daT, qT, cs, bc)
    _strip_covered_pe_waits(nc)
    _PROGRAM_CACHE[bc] = nc
    return nc


def kernel(**inputs):
    global LAST_RESULTS
    states = np.ascontiguousarray(np.asarray(inputs["states"], np.float32))
    assert states.shape == (B, 13), states.shape
    C = build_consts(inputs)
    dah = np.zeros((64, B), np.float32)
    dah[0:13] = states.T
    acts = states[:, 0].astype(np.int64)
    dah[32 + np.clip(acts, 0, 7), np.arange(B)] = 1.0  # onehot(act)

    nc = build_program(BC)
    in_maps = []
    for core in range(NCORES):
        m = dict(C)
        m["daT"] = round_f32r(dah[:, core * BC:(core + 1) * BC])
        in_maps.append(m)
    res = run_bass_kernel_spmd(
        nc, in_maps, core_ids=list(range(NCORES)),
        trace=bool(os.environ.get("FRAP_TRACE")),
    )
    LAST_RESULTS = res
    q = np.concatenate([r_["qT"] for r_ in res.results], axis=1).T
    return np.ascontiguousarray(q, np.float32)


if __name__ == "__main__":
    rng = np.random.default_rng(0)
    fake = dict(
        states=np.concatenate(
            [rng.integers(0, 8, (B, 1)).astype(np.float32),
             rng.random((B, 12), np.float32)], axis=1),
        phase2movements=rng.integers(0, 2, (8, 12)),
        oshape=np.int64(8),
        comp_mask=rng.integers(0, 2, (8, 7)),
        p_emb=rng.standard_normal((2, 4), np.float32) * 0.1,
        d_W=rng.standard_normal((4, 1), np.float32) * 0.1,
        d_b=rng.standard_normal((4,), np.float32) * 0.1,
        lane_W=rng.standard_normal((16, 8), np.float32) * 0.1,
        lane_b=rng.standard_normal((16,), np.float32) * 0.1,
        lane_conv_W=rng.standard_normal((20, 32), np.float32) * 0.1,
        lane_conv_b=rng.standard_normal((20,), np.float32) * 0.1,
        rel_emb=rng.standard_normal((2, 4), np.float32) * 0.1,
        rel_conv_W=rng.standard_normal((20, 4), np.float32) * 0.1,
        rel_conv_b=rng.standard_normal((20,), np.float32) * 0.1,
        hid_W=rng.standard_normal((20, 20), np.float32) * 0.1,
        hid_b=rng.standard_normal((20,), np.float32) * 0.1,
        merge_W=rng.standard_normal((1, 20), np.float32) * 0.1,
        merge_b=rng.standard_normal((1,), np.float32) * 0.1,
    )
    out = kernel(**fake)
    print("kernel output", out.shape, out.dtype)


# revision 27
# speedup vs baseline: 1.1065x; 1.0074x over previous
"""Trainium2 Bass kernel for nn_FRAP_move (FRAP traffic-signal Q-network).

Strategy
--------
Pure data parallelism over the batch dim (8 cores x 8192 rows). On each core
everything is computed feature-major: features live on SBUF partitions, a
batch tile of T=512 rows is the moving free dimension of every matmul.

All network parameters are tiny, and phase2movements / comp_mask are 0/1
masks fixed across the batch, so the whole [B,P,M,*] computation collapses
on the host into a handful of structured matrices that are applied on-device
as TensorE matmuls in float32r (fp32 bits, ~12-bit mantissa PE mode; streams
at ~2 cycles/column but keeps rel err ~6e-4 end to end).

The input daT[40, bc] carries states^T in rows 0..12 and a host-computed
onehot(act) in rows 32..39 (base-32 aligned for matmul operand slicing):

  daT --MM-A--> dW[k]*dem[m] --sigmoid+bias--> s1[48,T]
  s1,oh --MM-D (PSUM accum)--> pre[(m,h) 192,T] --relu--> relu1
  relu1,oh --MM-F (PSUM accum)--> agg[(p,h) 128,T]
  agg --MM-G--> rot_pre[(pair,o) 120,T] per 6-pair group --relu+bias (DVE)-->
      --MM-I (block-diag hid_W*rel)--> --relu+bias (ACT)--> --MM-J--> q[8,T]

The pairwise relation factor rel[i,j] takes only two values (comp_mask is
0/1), folded into the MM-I weights on the host.
"""

import os
import sys
from contextlib import ExitStack

import numpy as np

for _p in ("/opt/trn_rl_repo", "/root/.axon_site/_ro/trn_rl_repo"):
    if os.path.isdir(_p) and _p not in sys.path:
        sys.path.append(_p)

import concourse.bass as bass
import concourse.mybir as mybir
import concourse.tile as tile
from concourse.bass_utils import run_bass_kernel_spmd

F32 = mybir.dt.float32
F32R = mybir.dt.float32r
BF16 = mybir.dt.bfloat16
AF = mybir.ActivationFunctionType
ALU = mybir.AluOpType

B = 65536
NCORES = 8
BC = B // NCORES  # 8192 per core
T = 512           # batch tile (matmul moving free dim)

PAIRS = [(i, j) for i in range(8) for j in range(8) if j != i]
GROUPS = [PAIRS[g * 6:(g + 1) * 6] for g in range(9)] + [PAIRS[54:]]
GROUP_ROWS = [len(g) * 20 for g in GROUPS]           # [120]*9 + [40]
GROUP_OFF = np.cumsum([0] + GROUP_ROWS).tolist()     # offsets into 1120

CONST_SHAPES = {
    "cLA": (13, 48),
    "cDB": (48, 1),
    "cLD": (112, 192),
    "cLFLO": (96, 128),
    "cLFHI": (96, 128),
    "cLFOH": (40, 128),
    "cLG": (128, 1120),
    "cLI": (120, 1120),
    "cLJ": (120, 80),
    "cLCB": (120, 1),
    "cHB": (120, 1),
    "cQB": (8, 1),
}
# matmul operands live in float32r (PE full-rate fp32 mode, ~12 mantissa bits)
F32R_CONSTS = {"cLA", "cLD", "cLFLO", "cLFHI", "cLFOH",
               "cLG", "cLI", "cLJ"}
BF16_CONSTS = set()


def round_f32r(a):
    """Round fp32 array to the fp32r grid (12-bit mantissa, round-to-nearest)."""
    u = np.ascontiguousarray(a, np.float32).view(np.uint32)
    r = ((u.astype(np.uint64) + 0x800) & 0xFFFFF000).astype(np.uint32)
    return r.view(np.float32)

LAST_RESULTS = None
_PROGRAM_CACHE = {}


def _sigmoid(x):
    return 1.0 / (1.0 + np.exp(-x))


def _relu(x):
    return np.maximum(x, 0.0)


def build_consts(inputs):
    """Host-side fold of all parameters into the structured device matrices."""
    f32 = np.float32
    inp = {k: np.asarray(v) for k, v in inputs.items()}
    dW = inp["d_W"].astype(f32)[:, 0]
    db = inp["d_b"].astype(f32)
    lane_W = inp["lane_W"].astype(f32)
    lane_b = inp["lane_b"].astype(f32)
    Wd, We = lane_W[:, :4], lane_W[:, 4:]
    p_emb = inp["p_emb"].astype(f32)
    e0, e1 = _sigmoid(p_emb[0]), _sigmoid(p_emb[1])
    v0, v1 = We @ e0, We @ e1
    dv = v1 - v0
    u0 = Wd @ _sigmoid(db)
    r0 = _relu(u0 + v0 + lane_b)
    r1 = _relu(u0 + v1 + lane_b)
    drr = r1 - r0
    p2m = inp["phase2movements"].astype(f32)
    np_p = p2m.sum(1)
    lane_conv_W = inp["lane_conv_W"].astype(f32)
    W1, W2 = lane_conv_W[:, :16], lane_conv_W[:, 16:]
    lcb = inp["lane_conv_b"].astype(f32)
    relv = [
        _relu(inp["rel_conv_W"].astype(f32) @ _relu(inp["rel_emb"].astype(f32)[k])
              + inp["rel_conv_b"].astype(f32))
        for k in (0, 1)
    ]
    hid_W = inp["hid_W"].astype(f32)
    H = [hid_W * relv[k][None, :] for k in (0, 1)]
    hb = inp["hid_b"].astype(f32)
    mW = inp["merge_W"].astype(f32)[0]
    mb = float(inp["merge_b"].astype(f32)[0])
    comp = inp["comp_mask"].astype(np.int64)

    C = {}
    # MM-A: da[13,T] (row0=act, rows1..12=dem) -> dW[k]*dem[m] packed (k,m)
    LA = np.zeros((13, 48), f32)
    for k in range(4):
        for m in range(12):
            LA[1 + m, k * 12 + m] = dW[k]
    C["cLA"] = LA
    dbc = np.zeros((48, 1), f32)
    for k in range(4):
        dbc[k * 12:(k + 1) * 12, 0] = db[k]
    C["cDB"] = dbc

    # MM-D: s1x[56,T] = [s1 (k,m); onehot(8)] -> pre[(m,h) 192]
    LB = np.zeros((8, 13), f32)
    LB[:, :12] = p2m
    LB[:, 12] = 1.0
    LD_s1 = np.zeros((48, 192), f32)
    for k in range(4):
        for m in range(12):
            LD_s1[k * 12 + m, m * 16:(m + 1) * 16] = Wd[:, k]
    LD_c = np.zeros((13, 192), f32)
    for m in range(12):
        LD_c[m, m * 16:(m + 1) * 16] = dv
    LD_c[12, :] = np.tile(v0 + lane_b, 12)
    # one fused MM-D operand: rows 32..39 take the onehot weights, rows
    # 64..111 the sigmoid-block weights; rhs is the da tile itself, into
    # which the sigmoid writes at partition 64
    CLD = np.zeros((112, 192), f32)
    CLD[32:40] = LB @ LD_c
    CLD[64:112] = LD_s1
    C["cLD"] = CLD

    # MM-F: relu1[(m,h) 192] + onehot -> agg[(p,h) 128]
    LF_relu = np.zeros((192, 128), f32)
    for m in range(12):
        for p in range(8):
            if p2m[p, m] > 0.5:
                for h in range(16):
                    LF_relu[m * 16 + h, p * 16 + h] = 1.0
    LF_c = np.zeros((13, 128), f32)
    for p in range(8):
        for m in range(12):
            LF_c[m, p * 16:(p + 1) * 16] = (1.0 - p2m[p, m]) * drr
        LF_c[12, p * 16:(p + 1) * 16] = (12.0 - np_p[p]) * r0
    C["cLFLO"] = LF_relu[:96].copy()
    C["cLFHI"] = LF_relu[96:].copy()
    CLFOH = np.zeros((40, 128), f32)
    CLFOH[32:40] = LB @ LF_c
    C["cLFOH"] = CLFOH

    # pair stage
    LG = np.zeros((128, 1120), f32)
    LI = np.zeros((120, 1120), f32)
    LJ = np.zeros((120, 80), f32)
    for g, gp in enumerate(GROUPS):
        off = GROUP_OFF[g]
        for kk, (i, j) in enumerate(gp):
            col0 = off + kk * 20
            LG[i * 16:(i + 1) * 16, col0:col0 + 20] += W1.T
            LG[j * 16:(j + 1) * 16, col0:col0 + 20] += W2.T
            jj = [x for x in range(8) if x != i].index(j)
            mk = int(comp[i, jj])
            LI[kk * 20:(kk + 1) * 20, col0:col0 + 20] = H[mk].T
            LJ[kk * 20:(kk + 1) * 20, g * 8 + i] = mW
    C["cLG"] = LG
    C["cLI"] = LI
    C["cLJ"] = LJ
    C["cLCB"] = np.tile(lcb, 6)[:, None].astype(f32)
    C["cHB"] = np.tile(hb, 6)[:, None].astype(f32)
    C["cQB"] = np.full((8, 1), 7.0 * mb, f32)
    import ml_dtypes
    for k, v in C.items():
        assert v.shape == CONST_SHAPES[k], (k, v.shape)
        if k in F32R_CONSTS:
            C[k] = round_f32r(v)
        elif k in BF16_CONSTS:
            C[k] = np.ascontiguousarray(v.astype(ml_dtypes.bfloat16))
        else:
            C[k] = np.ascontiguousarray(v, f32)
    return C


def _emit(nc, tc, ctx, daT, qT, cs, bc):
    """Emit the per-core program: bc batch rows in tiles of T."""
    nt = bc // T
    ts = bass.ts

    consts = ctx.enter_context(tc.tile_pool(name="consts", bufs=1))
    sb = ctx.enter_context(tc.tile_pool(name="sb", bufs=3))
    sbp = ctx.enter_context(tc.tile_pool(name="sbp", bufs=3))
    ps1 = ctx.enter_context(tc.tile_pool(name="ps1", bufs=1, space="PSUM"))
    ps2 = ctx.enter_context(tc.tile_pool(name="ps2", bufs=2, space="PSUM"))

    c = {}
    for name, shape in CONST_SHAPES.items():
        dt_ = (F32R if name in F32R_CONSTS
               else BF16 if name in BF16_CONSTS else F32)
        t_ = consts.tile(list(shape), dt_, tag=name)
        nc.sync.dma_start(t_[:], cs[name].ap())
        c[name] = t_

    for t in range(nt):
        da = sb.tile([112, T], F32R, tag="da")
        nc.sync.dma_start(da[0:64, :], daT.ap()[:, ts(t, T)])

        ps48 = ps2.tile([48, T], F32, tag="ps_misc")
        nc.tensor.matmul(ps48[:], c["cLA"][:], da[0:13, :], start=True, stop=True)
        nc.scalar.activation(da[64:112, :], ps48[:], AF.Sigmoid,
                             bias=c["cDB"][:])
        oh = da[32:40, :]  # host-computed onehot rows

        pre_lo = ps2.tile([96, T], F32, tag="ps_misc")
        nc.tensor.matmul(pre_lo[:], c["cLD"][:, 0:96], da[0:112, :],
                         start=True, stop=True)
        pre_hi = ps2.tile([96, T], F32, tag="ps_misc")
        nc.tensor.matmul(pre_hi[:], c["cLD"][:, 96:192], da[0:112, :],
                         start=True, stop=True)
        r1lo = sb.tile([96, T], F32R, tag="r1lo")
        nc.scalar.activation(r1lo[:], pre_lo[:], AF.Relu)
        r1hi = sb.tile([96, T], F32R, tag="r1hi")
        nc.scalar.activation(r1hi[:], pre_hi[:], AF.Relu)

        ps_agg = ps1.tile([128, T], F32, tag="ps_agg")
        nc.tensor.matmul(ps_agg[:], c["cLFLO"][:], r1lo[:],
                         start=True, stop=False)
        nc.tensor.matmul(ps_agg[:], c["cLFHI"][:], r1hi[:],
                         start=False, stop=False)
        nc.tensor.matmul(ps_agg[:], c["cLFOH"][32:40, :], oh,
                         start=False, stop=True)
        agg = sb.tile([128, T], F32R, tag="agg")
        nc.vector.tensor_copy(agg[:], ps_agg[:])

        ps_q = ps1.tile([8, T], F32, tag="ps_q")
        for g in range(10):
            rows = GROUP_ROWS[g]
            off = GROUP_OFF[g]
            ps_rot = ps2.tile([120, T], F32, tag="ps_rot")
            nc.tensor.matmul(ps_rot[0:rows, :], c["cLG"][:, off:off + rows],
                             agg[:], start=True, stop=True)
            rot = sbp.tile([120, T], F32R, tag="rot")
            nc.vector.tensor_scalar(rot[0:rows, :], ps_rot[0:rows, :],
                                    c["cLCB"][0:rows, :], 0.0, ALU.add, ALU.max)
            ps_comb = ps2.tile([120, T], F32, tag="ps_comb")
            nc.tensor.matmul(ps_comb[0:rows, :],
                             c["cLI"][0:rows, off:off + rows],
                             rot[0:rows, :], start=True, stop=True)
            comb = sbp.tile([120, T], F32R, tag="comb")
            nc.scalar.activation(comb[0:rows, :], ps_comb[0:rows, :], AF.Relu,
                                 bias=c["cHB"][0:rows, :])
            nc.tensor.matmul(ps_q[:], c["cLJ"][0:rows, g * 8:(g + 1) * 8],
                             comb[0:rows, :], start=(g == 0), stop=(g == 9),
                             skip_group_check=True)

        q = sb.tile([8, T], F32, tag="q")
        nc.scalar.activation(q[:], ps_q[:], AF.Identity, bias=c["cQB"][:])
        nc.sync.dma_start(qT.ap()[:, ts(t, T)], q[:])


def _strip_covered_pe_waits(nc):
    """fp32r matmuls lower to a single fused instruction that can carry only
    ONE sync wait. Tile sometimes emits a PE self-wait (psum-bank WAW)
    alongside a compute-engine wait that already transitively guarantees it
    (Tile's vector clock is not transitive across engines). Strip a matmul's
    PE wait only when another of its waits provably implies it; fail loudly
    if any matmul still carries more than one wait."""
    from collections import defaultdict

    f = nc.m.functions[0]
    sem_instrs = defaultdict(list)  # sem name -> [(cum_value_after, pe_req)]
    cum = defaultdict(int)
    for blk in f.blocks:
        for ins in blk.instructions:
            si = ins.sync_info
            if si is None:
                continue
            pe_req = 0
            for w in si.on_wait:
                if w.ant_name and w.ant_name.startswith("PE"):
                    pe_req = max(pe_req, w.wait_value)
            for u in si.on_update:
                cum[u.ant_name] += u.update_value
                sem_instrs[u.ant_name].append((cum[u.ant_name], pe_req))
    prefix = {}
    for name, lst in sem_instrs.items():
        mx = 0
        out = []
        for cv, pr in lst:
            mx = max(mx, pr)
            out.append((cv, mx))
        prefix[name] = out

    def covered(sem, val, pe_needed):
        best = 0
        for cv, mx in prefix.get(sem, []):
            if cv <= val:
                best = mx
            else:
                break
        return best >= pe_needed

    bad = []
    for blk in f.blocks:
        for ins in blk.instructions:
            if "Matmult" not in type(ins).__name__:
                continue
            si = ins.sync_info
            if si is None or len(si.on_wait) < 2:
                continue
            pe_w = [w for w in si.on_wait if w.ant_name and w.ant_name.startswith("PE")]
            others = [w for w in si.on_wait if not (w.ant_name and w.ant_name.startswith("PE"))]
            if pe_w and others:
                need = max(w.wait_value for w in pe_w)
                if any(covered(w.ant_name, w.wait_value, need) for w in others):
                    si.on_wait = others
                    ins.sync_info = si
            si = ins.sync_info
            if len(si.on_wait) > 1:
                bad.append((ins.name, [w.ant_name for w in si.on_wait]))
    if bad:
        raise RuntimeError(f"matmuls with >1 sync wait (fp32r cap): {bad[:5]}")


def build_program(bc=BC):
    if bc in _PROGRAM_CACHE:
        return _PROGRAM_CACHE[bc]
    nc = bass.Bass("TRN2", target_bir_lowering=False, debug=False)
    cs = {name: nc.dram_tensor(name, list(shape),
                               F32R if name in F32R_CONSTS
                               else BF16 if name in BF16_CONSTS else F32,
                               kind="ExternalInput")
          for name, shape in CONST_SHAPES.items()}
    daT = nc.dram_tensor("daT", [64, bc], F32R, kind="ExternalInput")
    qT = nc.dram_tensor("qT", [8, bc], F32, kind="ExternalOutput")
    with tile.TileContext(nc) as tc, ExitStack() as ctx:
        _emit(nc, tc, ctx, daT, qT, cs, bc)
    _strip_covered_pe_waits(nc)
    _PROGRAM_CACHE[bc] = nc
    return nc


def kernel(**inputs):
    global LAST_RESULTS
    states = np.ascontiguousarray(np.asarray(inputs["states"], np.float32))
    assert states.shape == (B, 13), states.shape
    C = build_consts(inputs)
    dah = np.zeros((64, B), np.float32)
    dah[0:13] = states.T
    acts = states[:, 0].astype(np.int64)
    dah[32 + np.clip(acts, 0, 7), np.arange(B)] = 1.0  # onehot(act)

    nc = build_program(BC)
    in_maps = []
    for core in range(NCORES):
        m = dict(C)
        m["daT"] = round_f32r(dah[:, core * BC:(core + 1) * BC])
        in_maps.append(m)
    res = run_bass_kernel_spmd(
        nc, in_maps, core_ids=list(range(NCORES)),
        trace=bool(os.environ.get("FRAP_TRACE")),
    )
    LAST_RESULTS = res
    q = np.concatenate([r_["qT"] for r_ in res.results], axis=1).T
    return np.ascontiguousarray(q, np.float32)


if __name__ == "__main__":
    rng = np.random.default_rng(0)
    fake = dict(
        states=np.concatenate(
            [rng.integers(0, 8, (B, 1)).astype(np.float32),
             rng.random((B, 12), np.float32)], axis=1),
        phase2movements=rng.integers(0, 2, (8, 12)),
        oshape=np.int64(8),
        comp_mask=rng.integers(0, 2, (8, 7)),
        p_emb=rng.standard_normal((2, 4), np.float32) * 0.1,
        d_W=rng.standard_normal((4, 1), np.float32) * 0.1,
        d_b=rng.standard_normal((4,), np.float32) * 0.1,
        lane_W=rng.standard_normal((16, 8), np.float32) * 0.1,
        lane_b=rng.standard_normal((16,), np.float32) * 0.1,
        lane_conv_W=rng.standard_normal((20, 32), np.float32) * 0.1,
        lane_conv_b=rng.standard_normal((20,), np.float32) * 0.1,
        rel_emb=rng.standard_normal((2, 4), np.float32) * 0.1,
        rel_conv_W=rng.standard_normal((20, 4), np.float32) * 0.1,
        rel_conv_b=rng.standard_normal((20,), np.float32) * 0.1,
        hid_W=rng.standard_normal((20, 20), np.float32) * 0.1,
        hid_b=rng.standard_normal((20,), np.float32) * 0.1,
        merge_W=rng.standard_normal((1, 20), np.float32) * 0.1,
        merge_b=rng.standard_normal((1,), np.float32) * 0.1,
    )
    out = kernel(**fake)
    print("kernel output", out.shape, out.dtype)


# revision 28
# speedup vs baseline: 1.1408x; 1.0310x over previous
"""Trainium2 Bass kernel for nn_FRAP_move (FRAP traffic-signal Q-network).

Strategy
--------
Pure data parallelism over the batch dim (8 cores x 8192 rows). On each core
everything is computed feature-major: features live on SBUF partitions, a
batch tile of T=512 rows is the moving free dimension of every matmul.

All network parameters are tiny, and phase2movements / comp_mask are 0/1
masks fixed across the batch, so the whole [B,P,M,*] computation collapses
on the host into a handful of structured matrices that are applied on-device
as TensorE matmuls in float32r (fp32 bits, ~12-bit mantissa PE mode; streams
at ~2 cycles/column but keeps rel err ~6e-4 end to end).

The input daT[40, bc] carries states^T in rows 0..12 and a host-computed
onehot(act) in rows 32..39 (base-32 aligned for matmul operand slicing):

  daT --MM-A--> dW[k]*dem[m] --sigmoid+bias--> s1[48,T]
  s1,oh --MM-D (PSUM accum)--> pre[(m,h) 192,T] --relu--> relu1
  relu1,oh --MM-F (PSUM accum)--> agg[(p,h) 128,T]
  agg --MM-G--> rot_pre[(pair,o) 120,T] per 6-pair group --relu+bias (DVE)-->
      --MM-I (block-diag hid_W*rel)--> --relu+bias (ACT)--> --MM-J--> q[8,T]

The pairwise relation factor rel[i,j] takes only two values (comp_mask is
0/1), folded into the MM-I weights on the host.
"""

import os
import sys
from contextlib import ExitStack

import numpy as np

for _p in ("/opt/trn_rl_repo", "/root/.axon_site/_ro/trn_rl_repo"):
    if os.path.isdir(_p) and _p not in sys.path:
        sys.path.append(_p)

import concourse.bass as bass
import concourse.mybir as mybir
import concourse.tile as tile
from concourse.bass_utils import run_bass_kernel_spmd

F32 = mybir.dt.float32
F32R = mybir.dt.float32r
BF16 = mybir.dt.bfloat16
AF = mybir.ActivationFunctionType
ALU = mybir.AluOpType

B = 65536
NCORES = 8
BC = B // NCORES  # 8192 per core
T = 512           # batch tile (matmul moving free dim)

PAIRS = [(i, j) for i in range(8) for j in range(8) if j != i]
GROUPS = [PAIRS[g * 6:(g + 1) * 6] for g in range(9)] + [PAIRS[54:]]
GROUP_ROWS = [len(g) * 20 for g in GROUPS]           # [120]*9 + [40]
GROUP_OFF = np.cumsum([0] + GROUP_ROWS).tolist()     # offsets into 1120

CONST_SHAPES = {
    "cLA": (13, 48),
    "cDB": (48, 1),
    "cLD": (112, 192),
    "cLF1": (104, 128),
    "cLFHI": (96, 128),
    "cLG": (128, 1120),
    "cLI": (120, 1120),
    "cLJ": (120, 80),
    "cLCB": (120, 1),
    "cHB": (120, 1),
    "cQB": (8, 1),
}
# matmul operands live in float32r (PE full-rate fp32 mode, ~12 mantissa bits)
F32R_CONSTS = {"cLA", "cLD", "cLF1", "cLFHI",
               "cLG", "cLI", "cLJ"}
BF16_CONSTS = set()


def round_f32r(a):
    """Round fp32 array to the fp32r grid (12-bit mantissa, round-to-nearest)."""
    u = np.ascontiguousarray(a, np.float32).view(np.uint32)
    r = ((u.astype(np.uint64) + 0x800) & 0xFFFFF000).astype(np.uint32)
    return r.view(np.float32)

LAST_RESULTS = None
_PROGRAM_CACHE = {}


def _sigmoid(x):
    return 1.0 / (1.0 + np.exp(-x))


def _relu(x):
    return np.maximum(x, 0.0)


def build_consts(inputs):
    """Host-side fold of all parameters into the structured device matrices."""
    f32 = np.float32
    inp = {k: np.asarray(v) for k, v in inputs.items()}
    dW = inp["d_W"].astype(f32)[:, 0]
    db = inp["d_b"].astype(f32)
    lane_W = inp["lane_W"].astype(f32)
    lane_b = inp["lane_b"].astype(f32)
    Wd, We = lane_W[:, :4], lane_W[:, 4:]
    p_emb = inp["p_emb"].astype(f32)
    e0, e1 = _sigmoid(p_emb[0]), _sigmoid(p_emb[1])
    v0, v1 = We @ e0, We @ e1
    dv = v1 - v0
    u0 = Wd @ _sigmoid(db)
    r0 = _relu(u0 + v0 + lane_b)
    r1 = _relu(u0 + v1 + lane_b)
    drr = r1 - r0
    p2m = inp["phase2movements"].astype(f32)
    np_p = p2m.sum(1)
    lane_conv_W = inp["lane_conv_W"].astype(f32)
    W1, W2 = lane_conv_W[:, :16], lane_conv_W[:, 16:]
    lcb = inp["lane_conv_b"].astype(f32)
    relv = [
        _relu(inp["rel_conv_W"].astype(f32) @ _relu(inp["rel_emb"].astype(f32)[k])
              + inp["rel_conv_b"].astype(f32))
        for k in (0, 1)
    ]
    hid_W = inp["hid_W"].astype(f32)
    H = [hid_W * relv[k][None, :] for k in (0, 1)]
    hb = inp["hid_b"].astype(f32)
    mW = inp["merge_W"].astype(f32)[0]
    mb = float(inp["merge_b"].astype(f32)[0])
    comp = inp["comp_mask"].astype(np.int64)

    C = {}
    # MM-A: da[13,T] (row0=act, rows1..12=dem) -> dW[k]*dem[m] packed (k,m)
    LA = np.zeros((13, 48), f32)
    for k in range(4):
        for m in range(12):
            LA[1 + m, k * 12 + m] = dW[k]
    C["cLA"] = LA
    dbc = np.zeros((48, 1), f32)
    for k in range(4):
        dbc[k * 12:(k + 1) * 12, 0] = db[k]
    C["cDB"] = dbc

    # MM-D: s1x[56,T] = [s1 (k,m); onehot(8)] -> pre[(m,h) 192]
    LB = np.zeros((8, 13), f32)
    LB[:, :12] = p2m
    LB[:, 12] = 1.0
    LD_s1 = np.zeros((48, 192), f32)
    for k in range(4):
        for m in range(12):
            LD_s1[k * 12 + m, m * 16:(m + 1) * 16] = Wd[:, k]
    LD_c = np.zeros((13, 192), f32)
    for m in range(12):
        LD_c[m, m * 16:(m + 1) * 16] = dv
    LD_c[12, :] = np.tile(v0 + lane_b, 12)
    # one fused MM-D operand: rows 32..39 take the onehot weights, rows
    # 64..111 the sigmoid-block weights; rhs is the da tile itself, into
    # which the sigmoid writes at partition 64
    CLD = np.zeros((112, 192), f32)
    CLD[32:40] = LB @ LD_c
    CLD[64:112] = LD_s1
    C["cLD"] = CLD

    # MM-F: relu1[(m,h) 192] + onehot -> agg[(p,h) 128]
    LF_relu = np.zeros((192, 128), f32)
    for m in range(12):
        for p in range(8):
            if p2m[p, m] > 0.5:
                for h in range(16):
                    LF_relu[m * 16 + h, p * 16 + h] = 1.0
    LF_c = np.zeros((13, 128), f32)
    for p in range(8):
        for m in range(12):
            LF_c[m, p * 16:(p + 1) * 16] = (1.0 - p2m[p, m]) * drr
        LF_c[12, p * 16:(p + 1) * 16] = (12.0 - np_p[p]) * r0
    # r1lo tile carries onehot rows at partitions 96..103 (written by DMA),
    # so the onehot contribution rides the first MM-F matmul
    CLF1 = np.zeros((104, 128), f32)
    CLF1[0:96] = LF_relu[:96]
    CLF1[96:104] = LB @ LF_c
    C["cLF1"] = CLF1
    C["cLFHI"] = LF_relu[96:].copy()

    # pair stage
    LG = np.zeros((128, 1120), f32)
    LI = np.zeros((120, 1120), f32)
    LJ = np.zeros((120, 80), f32)
    for g, gp in enumerate(GROUPS):
        off = GROUP_OFF[g]
        for kk, (i, j) in enumerate(gp):
            col0 = off + kk * 20
            LG[i * 16:(i + 1) * 16, col0:col0 + 20] += W1.T
            LG[j * 16:(j + 1) * 16, col0:col0 + 20] += W2.T
            jj = [x for x in range(8) if x != i].index(j)
            mk = int(comp[i, jj])
            LI[kk * 20:(kk + 1) * 20, col0:col0 + 20] = H[mk].T
            LJ[kk * 20:(kk + 1) * 20, g * 8 + i] = mW
    C["cLG"] = LG
    C["cLI"] = LI
    C["cLJ"] = LJ
    C["cLCB"] = np.tile(lcb, 6)[:, None].astype(f32)
    C["cHB"] = np.tile(hb, 6)[:, None].astype(f32)
    C["cQB"] = np.full((8, 1), 7.0 * mb, f32)
    import ml_dtypes
    for k, v in C.items():
        assert v.shape == CONST_SHAPES[k], (k, v.shape)
        if k in F32R_CONSTS:
            C[k] = round_f32r(v)
        elif k in BF16_CONSTS:
            C[k] = np.ascontiguousarray(v.astype(ml_dtypes.bfloat16))
        else:
            C[k] = np.ascontiguousarray(v, f32)
    return C


def _emit(nc, tc, ctx, daT, qT, cs, bc):
    """Emit the per-core program: bc batch rows in tiles of T."""
    nt = bc // T
    ts = bass.ts

    consts = ctx.enter_context(tc.tile_pool(name="consts", bufs=1))
    sb = ctx.enter_context(tc.tile_pool(name="sb", bufs=3))
    sbp = ctx.enter_context(tc.tile_pool(name="sbp", bufs=3))
    ps1 = ctx.enter_context(tc.tile_pool(name="ps1", bufs=1, space="PSUM"))
    ps2 = ctx.enter_context(tc.tile_pool(name="ps2", bufs=2, space="PSUM"))

    c = {}
    for name, shape in CONST_SHAPES.items():
        dt_ = (F32R if name in F32R_CONSTS
               else BF16 if name in BF16_CONSTS else F32)
        t_ = consts.tile(list(shape), dt_, tag=name)
        nc.sync.dma_start(t_[:], cs[name].ap())
        c[name] = t_

    for t in range(nt):
        da = sb.tile([112, T], F32R, tag="da")
        nc.sync.dma_start(da[0:64, :], daT.ap()[:, ts(t, T)])

        ps48 = ps2.tile([48, T], F32, tag="ps_misc")
        nc.tensor.matmul(ps48[:], c["cLA"][:], da[0:13, :], start=True, stop=True)
        nc.scalar.activation(da[64:112, :], ps48[:], AF.Sigmoid,
                             bias=c["cDB"][:])
        oh = da[32:40, :]  # host-computed onehot rows

        pre_lo = ps2.tile([96, T], F32, tag="ps_misc")
        nc.tensor.matmul(pre_lo[:], c["cLD"][:, 0:96], da[0:112, :],
                         start=True, stop=True)
        pre_hi = ps2.tile([96, T], F32, tag="ps_misc")
        nc.tensor.matmul(pre_hi[:], c["cLD"][:, 96:192], da[0:112, :],
                         start=True, stop=True)
        r1lo = sb.tile([104, T], F32R, tag="r1lo")
        nc.scalar.activation(r1lo[0:96, :], pre_lo[:], AF.Relu)
        nc.sync.dma_start(r1lo[96:104, :], daT.ap()[32:40, ts(t, T)])
        r1hi = sb.tile([96, T], F32R, tag="r1hi")
        nc.scalar.activation(r1hi[:], pre_hi[:], AF.Relu)

        ps_agg = ps1.tile([128, T], F32, tag="ps_agg")
        nc.tensor.matmul(ps_agg[:], c["cLF1"][:], r1lo[:],
                         start=True, stop=False)
        nc.tensor.matmul(ps_agg[:], c["cLFHI"][:], r1hi[:],
                         start=False, stop=True)
        agg = sb.tile([128, T], F32R, tag="agg")
        nc.vector.tensor_copy(agg[:], ps_agg[:])

        ps_q = ps1.tile([8, T], F32, tag="ps_q")
        for g in range(10):
            rows = GROUP_ROWS[g]
            off = GROUP_OFF[g]
            ps_rot = ps2.tile([120, T], F32, tag="ps_rot")
            nc.tensor.matmul(ps_rot[0:rows, :], c["cLG"][:, off:off + rows],
                             agg[:], start=True, stop=True)
            rot = sbp.tile([120, T], F32R, tag="rot")
            nc.vector.tensor_scalar(rot[0:rows, :], ps_rot[0:rows, :],
                                    c["cLCB"][0:rows, :], 0.0, ALU.add, ALU.max)
            ps_comb = ps2.tile([120, T], F32, tag="ps_comb")
            nc.tensor.matmul(ps_comb[0:rows, :],
                             c["cLI"][0:rows, off:off + rows],
                             rot[0:rows, :], start=True, stop=True)
            comb = sbp.tile([120, T], F32R, tag="comb")
            nc.scalar.activation(comb[0:rows, :], ps_comb[0:rows, :], AF.Relu,
                                 bias=c["cHB"][0:rows, :])
            nc.tensor.matmul(ps_q[:], c["cLJ"][0:rows, g * 8:(g + 1) * 8],
                             comb[0:rows, :], start=(g == 0), stop=(g == 9),
                             skip_group_check=True)

        q = sb.tile([8, T], F32, tag="q")
        nc.scalar.activation(q[:], ps_q[:], AF.Identity, bias=c["cQB"][:])
        nc.sync.dma_start(qT.ap()[:, ts(t, T)], q[:])


def _strip_covered_pe_waits(nc):
    """fp32r matmuls lower to a single fused instruction that can carry only
    ONE sync wait. Tile sometimes emits a PE self-wait (psum-bank WAW)
    alongside a compute-engine wait that already transitively guarantees it
    (Tile's vector clock is not transitive across engines). Strip a matmul's
    PE wait only when another of its waits provably implies it; fail loudly
    if any matmul still carries more than one wait."""
    from collections import defaultdict

    f = nc.m.functions[0]
    sem_instrs = defaultdict(list)  # sem name -> [(cum_value_after, pe_req)]
    cum = defaultdict(int)
    for blk in f.blocks:
        for ins in blk.instructions:
            si = ins.sync_info
            if si is None:
                continue
            pe_req = 0
            for w in si.on_wait:
                if w.ant_name and w.ant_name.startswith("PE"):
                    pe_req = max(pe_req, w.wait_value)
            for u in si.on_update:
                cum[u.ant_name] += u.update_value
                sem_instrs[u.ant_name].append((cum[u.ant_name], pe_req))
    prefix = {}
    for name, lst in sem_instrs.items():
        mx = 0
        out = []
        for cv, pr in lst:
            mx = max(mx, pr)
            out.append((cv, mx))
        prefix[name] = out

    def covered(sem, val, pe_needed):
        best = 0
        for cv, mx in prefix.get(sem, []):
            if cv <= val:
                best = mx
            else:
                break
        return best >= pe_needed

    bad = []
    for blk in f.blocks:
        for ins in blk.instructions:
            if "Matmult" not in type(ins).__name__:
                continue
            si = ins.sync_info
            if si is None or len(si.on_wait) < 2:
                continue
            pe_w = [w for w in si.on_wait if w.ant_name and w.ant_name.startswith("PE")]
            others = [w for w in si.on_wait if not (w.ant_name and w.ant_name.startswith("PE"))]
            if pe_w and others:
                need = max(w.wait_value for w in pe_w)
                if any(covered(w.ant_name, w.wait_value, need) for w in others):
                    si.on_wait = others
                    ins.sync_info = si
            si = ins.sync_info
            if len(si.on_wait) > 1:
                bad.append((ins.name, [w.ant_name for w in si.on_wait]))
    if bad:
        raise RuntimeError(f"matmuls with >1 sync wait (fp32r cap): {bad[:5]}")


def build_program(bc=BC):
    if bc in _PROGRAM_CACHE:
        return _PROGRAM_CACHE[bc]
    nc = bass.Bass("TRN2", target_bir_lowering=False, debug=False)
    cs = {name: nc.dram_tensor(name, list(shape),
                               F32R if name in F32R_CONSTS
                               else BF16 if name in BF16_CONSTS else F32,
                               kind="ExternalInput")
          for name, shape in CONST_SHAPES.items()}
    daT = nc.dram_tensor("daT", [64, bc], F32R, kind="ExternalInput")
    qT = nc.dram_tensor("qT", [8, bc], F32, kind="ExternalOutput")
    with tile.TileContext(nc) as tc, ExitStack() as ctx:
        _emit(nc, tc, ctx, daT, qT, cs, bc)
    _strip_covered_pe_waits(nc)
    _PROGRAM_CACHE[bc] = nc
    return nc


def kernel(**inputs):
    global LAST_RESULTS
    states = np.ascontiguousarray(np.asarray(inputs["states"], np.float32))
    assert states.shape == (B, 13), states.shape
    C = build_consts(inputs)
    dah = np.zeros((64, B), np.float32)
    dah[0:13] = states.T
    acts = states[:, 0].astype(np.int64)
    dah[32 + np.clip(acts, 0, 7), np.arange(B)] = 1.0  # onehot(act)

    nc = build_program(BC)
    in_maps = []
    for core in range(NCORES):
        m = dict(C)
        m["daT"] = round_f32r(dah[:, core * BC:(core + 1) * BC])
        in_maps.append(m)
    res = run_bass_kernel_spmd(
        nc, in_maps, core_ids=list(range(NCORES)),
        trace=bool(os.environ.get("FRAP_TRACE")),
    )
    LAST_RESULTS = res
    q = np.concatenate([r_["qT"] for r_ in res.results], axis=1).T
    return np.ascontiguousarray(q, np.float32)


if __name__ == "__main__":
    rng = np.random.default_rng(0)
    fake = dict(
        states=np.concatenate(
            [rng.integers(0, 8, (B, 1)).astype(np.float32),
             rng.random((B, 12), np.float32)], axis=1),
        phase2movements=rng.integers(0, 2, (8, 12)),
        oshape=np.int64(8),
        comp_mask=rng.integers(0, 2, (8, 7)),
        p_emb=rng.standard_normal((2, 4), np.float32) * 0.1,
        d_W=rng.standard_normal((4, 1), np.float32) * 0.1,
        d_b=rng.standard_normal((4,), np.float32) * 0.1,
        lane_W=rng.standard_normal((16, 8), np.float32) * 0.1,
        lane_b=rng.standard_normal((16,), np.float32) * 0.1,
        lane_conv_W=rng.standard_normal((20, 32), np.float32) * 0.1,
        lane_conv_b=rng.standard_normal((20,), np.float32) * 0.1,
        rel_emb=rng.standard_normal((2, 4), np.float32) * 0.1,
        rel_conv_W=rng.standard_normal((20, 4), np.float32) * 0.1,
        rel_conv_b=rng.standard_normal((20,), np.float32) * 0.1,
        hid_W=rng.standard_normal((20, 20), np.float32) * 0.1,
        hid_b=rng.standard_normal((20,), np.float32) * 0.1,
        merge_W=rng.standard_normal((1, 20), np.float32) * 0.1,
        merge_b=rng.standard_normal((1,), np.float32) * 0.1,
    )
    out = kernel(**fake)
    print("kernel output", out.shape, out.dtype)
